# revision 24
# baseline (speedup 1.0000x reference)
"""HAN kernel for Trainium2, 8-core data parallel (4 docs/core).

v2: PSUM-resident g_pre (no identity matmuls / no g_pre copies),
per-direction interleaved LSTM chains, gathers streamed under the LSTM,
batched weight DMAs.
"""
import os
import sys
sys.path.insert(0, "/opt/trn_rl_repo")
import numpy as np
import ml_dtypes

import concourse.bass as bass
import concourse.bacc as bacc
import concourse.mybir as mybir
import concourse.tile as tile
from concourse.bass_utils import run_bass_kernel_spmd

BF16 = mybir.dt.bfloat16
F32 = mybir.dt.float32
I32 = mybir.dt.int32
AF = mybir.ActivationFunctionType
ALU = mybir.AluOpType
AX = mybir.AxisListType

B, S, L = 32, 16, 64
V, E, P, H = 32000, 300, 100, 256
POL = 3
NCORES = 8
ND = B // NCORES          # 4 docs per core
NS = ND * S               # 64 sentences per core
NT = NS * L               # 4096 word tokens per core
NTILE = NT // 128         # 32 token tiles

# gate permutation [i,f,g,o] -> [i,f,o,g]
GPERM = np.concatenate([np.arange(0, 512), np.arange(768, 1024), np.arange(512, 768)])


def _bf(x):
    return np.asarray(x, dtype=np.float32).astype(ml_dtypes.bfloat16)


def _ap(base, off_delta, dims):
    return bass.AP(tensor=base.tensor, offset=base.offset + off_delta, ap=dims)


def _build_nc():
    nc = bacc.Bacc("TRN2", target_bir_lowering=False, debug=False, num_devices=NCORES)

    def din(name, shape, dt):
        return nc.declare_dram_parameter(name, list(shape), dt, isOutput=False)

    t = {}
    t["idx"] = din("idx", [128, 2, NTILE], I32)
    t["sp_idx"] = din("sp_idx", [NS, 1], I32)
    t["emb_t"] = din("emb_t", [V, E], BF16)
    t["wpe_t"] = din("wpe_t", [1005, P], BF16)
    t["spe_t"] = din("spe_t", [25, P], BF16)
    # all weights partition-major [128, ...]
    t["wihT"] = din("wihT", [128, 2, 4, 1024], BF16)
    t["whhT"] = din("whhT", [128, 2, 2, 1024], BF16)
    t["swihT"] = din("swihT", [128, 2, 5, 1024], BF16)
    t["swhhT"] = din("swhhT", [128, 2, 2, 1024], BF16)
    t["wordW"] = din("wordW", [128, 5, 640], BF16)
    t["wordb"] = din("wordb", [128, 5], F32)
    t["wproj"] = din("wproj", [128, 5], BF16)
    t["sentW"] = din("sentW", [128, 5, 640], BF16)
    t["sentb"] = din("sentb", [128, 5], F32)
    t["sproj"] = din("sproj", [128, 5], BF16)
    t["denseWT"] = din("denseWT", [128, 8, POL], BF16)
    t["denseb"] = din("denseb", [POL, 1], F32)
    t["id128"] = din("id128", [128, 128], BF16)
    t["m2"] = din("m2", [128, 64], BF16)
    t["y_out"] = nc.declare_dram_parameter("y", [ND, POL], F32, isOutput=True)
    if os.environ.get("HAN_DEBUG"):
        t["dbg_xe"] = nc.declare_dram_parameter("dbg_xe", [128, 3 * NT], BF16,
                                                isOutput=True)
        t["dbg_xp"] = nc.declare_dram_parameter("dbg_xp", [128, NT], BF16,
                                                isOutput=True)
        t["dbg_h"] = nc.declare_dram_parameter("dbg_h", [128, 4 * NT], BF16,
                                               isOutput=True)
    t["scores_d"] = nc.dram_tensor("scores_d", [NT], F32).ap()
    t["scs_d"] = nc.dram_tensor("scs_d", [NS], F32).ap()
    t["alps_d"] = nc.dram_tensor("alps_d", [ND, S], BF16).ap()

    with tile.TileContext(nc) as tc:
        _emit(nc, tc, t)
    nc.compile()
    return nc


def _emit(nc, tc, t):
    with tc.tile_pool(name="wgt", bufs=1) as wgt, \
         tc.tile_pool(name="big", bufs=1) as big, \
         tc.tile_pool(name="gath", bufs=5) as gath, \
         tc.tile_pool(name="small", bufs=2) as small, \
         tc.tile_pool(name="st", bufs=1) as st, \
         tc.tile_pool(name="pst", bufs=2, space="PSUM") as pst:

        # ================= prologue: constants + weights =================
        idx_sb = wgt.tile([128, 2, NTILE], I32)
        nc.sync.dma_start(out=idx_sb[:], in_=t["idx"][:])
        ident = wgt.tile([128, 128], BF16)
        nc.sync.dma_start(out=ident[:], in_=t["id128"][:])
        m2_sb = wgt.tile([128, 64], BF16)
        nc.sync.dma_start(out=m2_sb[:], in_=t["m2"][:])

        # ---- x^T buffers + memsets (early: transposes wait on these) ----
        xT_emb = big.tile([128, 3, NT], BF16, tag="xTe")
        xT_pos = big.tile([128, NT], BF16, tag="xTp")
        nc.gpsimd.memset(xT_emb[:, 2, :], 0.0)
        nc.gpsimd.memset(xT_pos[:], 0.0)
        nc.gpsimd.memset(xT_pos[96:128, :], 1.0)  # ones row -> bias via wihT row 511

        wih_sb = wgt.tile([128, 2, 4, 1024], BF16)
        for d in range(2):
            for k in range(4):
                nc.sync.dma_start(out=wih_sb[:, d, k, :], in_=t["wihT"][:, d, k, :])
        whh_sb = wgt.tile([128, 2, 2, 1024], BF16)
        for d in range(2):
            nc.sync.dma_start(out=whh_sb[:, d, :, :], in_=t["whhT"][:, d, :, :])

        def emit_gather(j):
            ge = gath.tile([128, E], BF16, tag="ge")
            gw = gath.tile([128, P], BF16, tag="gw")
            nc.gpsimd.indirect_dma_start(
                out=ge[:], out_offset=None, in_=t["emb_t"][:],
                in_offset=bass.IndirectOffsetOnAxis(ap=idx_sb[:, 0, j:j + 1], axis=0))
            nc.gpsimd.indirect_dma_start(
                out=gw[:], out_offset=None, in_=t["wpe_t"][:],
                in_offset=bass.IndirectOffsetOnAxis(ap=idx_sb[:, 1, j:j + 1], axis=0))
            return ge, gw

        def emit_transpose(j, ge, gw):
            cols = slice(j * 128, (j + 1) * 128)
            for c, (src, rows) in enumerate([(ge[:, 0:128], 128), (ge[:, 128:256], 128),
                                             (ge[:, 256:300], 44), (gw[:, :], 100)]):
                pt = pst.tile([128, 128], BF16, tag="pst")
                nc.tensor.transpose(out=pt[:rows, :], in_=src, identity=ident[:])
                dst = xT_pos[:rows, cols] if c == 3 else xT_emb[:rows, c, cols]
                if c % 2 == 0:
                    nc.vector.tensor_copy(out=dst, in_=pt[:rows, :])
                else:
                    nc.scalar.copy(out=dst, in_=pt[:rows, :])

        # tile pairs: pair p = tiles (p, 31-p); needed at step 2p.
        PAIRS = [(p, NTILE - 1 - p) for p in range(NTILE // 2)]
        pend = {}
        GLEAD, TLEAD = 5, 3    # pairs gathered / transposed ahead (in steps of 2)
        for p in range(min(GLEAD, len(PAIRS))):
            for j in PAIRS[p]:
                pend[j] = emit_gather(j)
        for p in range(min(TLEAD, len(PAIRS))):
            for j in PAIRS[p]:
                emit_transpose(j, *pend.pop(j))

        # sentence-level weights stream in during the LSTM
        swih_sb = wgt.tile([128, 2, 5, 1024], BF16)
        for d in range(2):
            for k in range(5):
                nc.sync.dma_start(out=swih_sb[:, d, k, :], in_=t["swihT"][:, d, k, :])
        swhh_sb = wgt.tile([128, 2, 2, 1024], BF16)
        for d in range(2):
            nc.sync.dma_start(out=swhh_sb[:, d, :, :], in_=t["swhhT"][:, d, :, :])
        wordW_sb = wgt.tile([128, 5, 640], BF16, tag="wordW")
        for k in range(5):
            nc.sync.dma_start(out=wordW_sb[:, k, :], in_=t["wordW"][:, k, :])
        wordb_sb = wgt.tile([128, 5], F32, tag="wordb")
        nc.sync.dma_start(out=wordb_sb[:], in_=t["wordb"][:])
        wproj_sb = wgt.tile([128, 5], BF16, tag="wproj")
        nc.sync.dma_start(out=wproj_sb[:], in_=t["wproj"][:])
        sentW_sb = wgt.tile([128, 5, 640], BF16, tag="sentW")
        for k in range(5):
            nc.sync.dma_start(out=sentW_sb[:, k, :], in_=t["sentW"][:, k, :])
        sentb_sb = wgt.tile([128, 5], F32, tag="sentb")
        nc.sync.dma_start(out=sentb_sb[:], in_=t["sentb"][:])
        sproj_sb = wgt.tile([128, 5], BF16, tag="sproj")
        nc.sync.dma_start(out=sproj_sb[:], in_=t["sproj"][:])
        dW_sb = wgt.tile([128, 8, POL], BF16)
        nc.sync.dma_start(out=dW_sb[:], in_=t["denseWT"][:])
        db_sb = wgt.tile([POL, 1], F32)
        nc.sync.dma_start(out=db_sb[:], in_=t["denseb"][:])
        si = gath.tile([NS, 1], I32, tag="si")
        nc.sync.dma_start(out=si[:], in_=t["sp_idx"][:])
        gs = gath.tile([NS, P], BF16, tag="gs")
        nc.gpsimd.indirect_dma_start(
            out=gs[:], out_offset=None, in_=t["spe_t"][:],
            in_offset=bass.IndirectOffsetOnAxis(ap=si[:, :1], axis=0))

        # ================= word-level BiLSTM =================
        h_all = big.tile([128, 4, NT], BF16, tag="h_all")
        c_st = [st.tile([128, 2, 64], F32, tag=f"c{d}", name=f"c{d}")
                for d in range(2)]

        def gpre_mm(d, gps, tau):
            # g_pre for step tau of direction d -> PSUM (start of accum group)
            tok0 = tau * 64 if d == 0 else (L - 1 - tau) * 64
            for m in range(8):
                for k in range(4):
                    rhs = xT_pos[:, tok0:tok0 + 64] if k == 3 else \
                        xT_emb[:, k, tok0:tok0 + 64]
                    # one start per 2KB psum zero-region (the whole gps bank):
                    # later first-touches are zeroed via the pending-zero bits.
                    nc.tensor.matmul(
                        out=gps[:, m, :], lhsT=wih_sb[:, d, k, m * 128:(m + 1) * 128],
                        rhs=rhs, start=(m == 0 and k == 0),
                        stop=(tau == 0 and m == 7 and k == 3),
                        skip_group_check=True)

        with tc.tile_pool(name="psg", bufs=2, space="PSUM") as psg:
            gps_cur = [None, None]
            for d in range(2):
                gps_cur[d] = psg.tile([128, 8, 64], F32, tag=f"g{d}",
                                      name=f"gps{d}")
                gpre_mm(d, gps_cur[d], 0)

            for tau in range(L):
                gps_nxt = [None, None]
                for d in range(2):
                    gps = gps_cur[d]
                    if tau > 0:
                        hcol = tau - 1 if d == 0 else L - tau
                        for k in range(2):
                            for m in range(8):
                                nc.tensor.matmul(
                                    out=gps[:, m, :],
                                    lhsT=whh_sb[:, d, k, m * 128:(m + 1) * 128],
                                    rhs=h_all[:, 2 * d + k, hcol * 64:(hcol + 1) * 64],
                                    start=False, stop=(k == 1 and m == 7),
                                    skip_group_check=True)
                    # prefetch next step's g_pre while this dir's elementwise runs
                    if tau + 1 < L:
                        gps_nxt[d] = psg.tile([128, 8, 64], F32, tag=f"g{d}",
                                              name=f"gpsn{d}")
                        gpre_mm(d, gps_nxt[d], tau + 1)

                    sig = small.tile([128, 6, 64], F32, tag=f"sig{d}")
                    tg = small.tile([128, 2, 64], F32, tag=f"tg{d}")
                    nc.scalar.activation(out=sig[:], in_=gps[:, 0:6, :], func=AF.Sigmoid)
                    nc.scalar.activation(out=tg[:], in_=gps[:, 6:8, :], func=AF.Tanh)
                    if tau == 0:
                        nc.vector.tensor_mul(out=c_st[d][:], in0=sig[:, 0:2, :], in1=tg[:])
                    else:
                        m1 = small.tile([128, 2, 64], F32, tag=f"m1{d}")
                        mm = small.tile([128, 2, 64], F32, tag=f"mm{d}")
                        nc.vector.tensor_mul(out=mm[:], in0=sig[:, 2:4, :], in1=c_st[d][:])
                        nc.vector.tensor_mul(out=m1[:], in0=sig[:, 0:2, :], in1=tg[:])
                        nc.vector.tensor_add(out=c_st[d][:], in0=m1[:], in1=mm[:])
                    tc_ = small.tile([128, 2, 64], F32, tag=f"tc{d}")
                    nc.scalar.activation(out=tc_[:], in_=c_st[d][:], func=AF.Tanh)
                    col = tau if d == 0 else L - 1 - tau
                    nc.vector.tensor_mul(
                        out=h_all[:, 2 * d:2 * d + 2, col * 64:(col + 1) * 64],
                        in0=sig[:, 4:6, :], in1=tc_[:])
                gps_cur = gps_nxt

                # stream gathers/transposes for later tile pairs
                if tau % 2 == 0:
                    p = tau // 2
                    if p + GLEAD < len(PAIRS):
                        for j in PAIRS[p + GLEAD]:
                            pend[j] = emit_gather(j)
                    if p + TLEAD < len(PAIRS):
                        for j in PAIRS[p + TLEAD]:
                            emit_transpose(j, *pend.pop(j))

        if "dbg_xe" in t:
            nc.sync.dma_start(out=t["dbg_xe"][:], in_=xT_emb[:])
            nc.sync.dma_start(out=t["dbg_xp"][:], in_=xT_pos[:])
            nc.sync.dma_start(out=t["dbg_h"][:], in_=h_all[:])

        # ================= word attention =================
        from contextlib import ExitStack
        ps_ctx = ExitStack()
        psA = ps_ctx.enter_context(tc.tile_pool(name="psA", bufs=2, space="PSUM"))
        h_batch = big.tile([128, 32, 512], BF16, tag="xTe")  # reuse xT_emb slot
        sc2 = small.tile([64, 64], F32, tag="sc2")
        for nb in range(8):
            cols = slice(nb * 512, (nb + 1) * 512)
            uT = small.tile([128, 5, 512], BF16, tag="uT")
            for m in range(5):
                pu = psA.tile([128, 512], F32, tag="pu")
                for k in range(5):
                    rhs = xT_pos[:, cols] if k == 4 else h_all[:, k, cols]
                    nc.tensor.matmul(
                        out=pu[:], lhsT=wordW_sb[:, k, m * 128:(m + 1) * 128],
                        rhs=rhs, start=(k == 0), stop=(k == 4))
                nc.scalar.activation(out=uT[:, m, :], in_=pu[:], func=AF.Tanh,
                                     bias=wordb_sb[:, m:m + 1])
            psc = psA.tile([1, 512], F32, tag="psc")
            for k in range(5):
                nc.tensor.matmul(out=psc[:], lhsT=wproj_sb[:, k:k + 1], rhs=uT[:, k, :],
                                start=(k == 0), stop=(k == 4), skip_group_check=True)
            scst = small.tile([1, 512], F32, tag="scst")
            nc.vector.tensor_copy(out=scst[:], in_=psc[:])
            nc.sync.dma_start(out=t["scores_d"][nb * 512:(nb + 1) * 512], in_=scst[:])
        nc.sync.dma_start(out=sc2[:], in_=_ap(t["scores_d"], 0, [[1, 64], [64, 64]]))
        # h^T transposes fill the PE while softmax/amat run on ACT/DVE.
        # Two transposes per psum tile -> one [128, 256] copy.
        for j in range(NTILE):
            for cc in range(2):
                pt = pst.tile([128, 256], BF16, tag="pst")
                for c in (2 * cc, 2 * cc + 1):
                    nc.tensor.transpose(out=pt[:, (c % 2) * 128:(c % 2) * 128 + 128],
                                        in_=h_all[:, c, j * 128:(j + 1) * 128],
                                        identity=ident[:])
                if (j + cc) % 2 == 0:
                    nc.vector.tensor_copy(
                        out=h_batch[:, j, cc * 256:(cc + 1) * 256], in_=pt[:])
                else:
                    nc.scalar.copy(
                        out=h_batch[:, j, cc * 256:(cc + 1) * 256], in_=pt[:])
        mx = small.tile([64, 1], F32, tag="mx")
        nc.vector.tensor_reduce(out=mx[:], in_=sc2[:], axis=AX.X, op=ALU.max, negate=True)
        ex = small.tile([64, 64], F32, tag="ex")
        den = small.tile([64, 1], F32, tag="den")
        nc.scalar.activation(out=ex[:], in_=sc2[:], func=AF.Exp, bias=mx[:],
                             accum_out=den[:])
        rcp = small.tile([64, 1], F32, tag="rcp")
        nc.vector.reciprocal(out=rcp[:], in_=den[:])
        alpha_bf = small.tile([64, 64], BF16, tag="alpha_bf")
        nc.vector.tensor_scalar_mul(out=alpha_bf[:], in0=ex[:], scalar1=rcp[:])
        alpha2 = small.tile([128, 64], BF16, tag="alpha2")
        nc.sync.dma_start(out=alpha2[0:64, :], in_=alpha_bf[:])
        nc.sync.dma_start(out=alpha2[64:128, :], in_=alpha_bf[:])
        # amat[p, j, s] = alpha[p%64, 2j + (p>=64)] * (p%64 == s)
        amat = big.tile([128, 32, 64], BF16, tag="amat")
        for half in range(2):
            pr = slice(half * 64, half * 64 + 64)
            nc.vector.tensor_tensor(
                out=amat[pr, :, :],
                in0=_ap(m2_sb[pr, :], 0, [[64, 64], [0, 32], [1, 64]]),
                in1=_ap(alpha2[pr, :], half, [[64, 64], [2, 32], [0, 64]]),
                op=ALU.mult)
        psen = psA.tile([64, 512], F32, tag="pu")
        for j in range(NTILE):
            nc.tensor.matmul(out=psen[:], lhsT=amat[:, j, :], rhs=h_batch[:, j, :],
                            start=(j == 0), stop=(j == NTILE - 1),
                            skip_group_check=True)
        sen_sb = st.tile([64, 512], BF16)
        nc.vector.tensor_copy(out=sen_sb[:], in_=psen[:])

        # ================= sentence level =================
        # sxT [128, 5, 64] doc-major cols (d*16 + sigma)
        sxT = st.tile([128, 5, NS], BF16)
        nc.gpsimd.memset(sxT[:, 4, :], 0.0)
        nc.gpsimd.memset(sxT[96:128, 4, :], 1.0)
        pt_s = pst.tile([128, 128], BF16, tag="pst")
        nc.tensor.transpose(out=pt_s[:P, :64], in_=gs[:], identity=ident[:64, :64])
        nc.vector.tensor_copy(out=sxT[:P, 4, :], in_=pt_s[:P, :64])
        for c in range(4):
            ptv = pst.tile([128, 128], BF16, tag="pst")
            nc.tensor.transpose(out=ptv[:, :64], in_=sen_sb[:, c * 128:(c + 1) * 128],
                                identity=ident[:64, :64])
            nc.vector.tensor_copy(out=sxT[:, c, :], in_=ptv[:, :64])

        hs_all = st.tile([128, 4, NS], BF16)
        cs_st = [st.tile([128, 2, ND], F32, tag=f"cs{d}", name=f"cs{d}")
                 for d in range(2)]

        with tc.tile_pool(name="psg2", bufs=1, space="PSUM") as psg2:
            # sentence g_pre, all 16 steps, PSUM-resident
            sgps = psg2.tile([128, 2, 8, NS], F32)
            for d in range(2):
                for m in range(8):
                    for k in range(5):
                        nc.tensor.matmul(
                            out=sgps[:, d, m, :],
                            lhsT=swih_sb[:, d, k, m * 128:(m + 1) * 128],
                            rhs=sxT[:, k, :], start=(m == 0 and k == 0), stop=False,
                            skip_group_check=True)

            for tau in range(S):
                for d in range(2):
                    pos = tau if d == 0 else S - 1 - tau
                    if tau > 0:
                        hpos = tau - 1 if d == 0 else S - tau
                        for k in range(2):
                            for m in range(8):
                                nc.tensor.matmul(
                                    out=_ap(sgps[:, d, m, :], pos,
                                            [[1024, 128], [16, ND]]),
                                    lhsT=swhh_sb[:, d, k, m * 128:(m + 1) * 128],
                                    rhs=_ap(hs_all[:, 2 * d + k, :], hpos,
                                            [[256, 128], [16, ND]]),
                                    start=False, stop=(k == 1 and m == 7),
                                    skip_group_check=True)
                    sigs = small.tile([128, 6, ND], F32, tag=f"ssig{d}")
                    tgs = small.tile([128, 2, ND], F32, tag=f"stg{d}")
                    nc.scalar.activation(
                        out=sigs[:], func=AF.Sigmoid,
                        in_=_ap(sgps[:, d, 0, :], pos, [[1024, 128], [64, 6], [16, ND]]))
                    nc.scalar.activation(
                        out=tgs[:], func=AF.Tanh,
                        in_=_ap(sgps[:, d, 6, :], pos, [[1024, 128], [64, 2], [16, ND]]))
                    if tau == 0:
                        nc.vector.tensor_mul(out=cs_st[d][:], in0=sigs[:, 0:2, :], in1=tgs[:])
                    else:
                        m1s = small.tile([128, 2, ND], F32, tag=f"sm1{d}")
                        mms = small.tile([128, 2, ND], F32, tag=f"smm{d}")
                        nc.vector.tensor_mul(out=mms[:], in0=sigs[:, 2:4, :], in1=cs_st[d][:])
                        nc.vector.tensor_mul(out=m1s[:], in0=sigs[:, 0:2, :], in1=tgs[:])
                        nc.vector.tensor_add(out=cs_st[d][:], in0=m1s[:], in1=mms[:])
                    tcs = small.tile([128, 2, ND], F32, tag=f"stc{d}")
                    nc.scalar.activation(out=tcs[:], in_=cs_st[d][:], func=AF.Tanh)
                    nc.vector.tensor_mul(
                        out=_ap(hs_all[:, 2 * d, :], pos, [[256, 128], [64, 2], [16, ND]]),
                        in0=sigs[:, 4:6, :], in1=tcs[:])

        # ---- sentence attention ----
        usT = st.tile([128, 5, NS], BF16)
        for m in range(5):
            pu = psA.tile([128, 64], F32, tag="pu")
            for k in range(5):
                rhs = sxT[:, 4, :] if k == 4 else hs_all[:, k, :]
                nc.tensor.matmul(out=pu[:], lhsT=sentW_sb[:, k, m * 128:(m + 1) * 128],
                                rhs=rhs, start=(k == 0), stop=(k == 4))
            nc.scalar.activation(out=usT[:, m, :], in_=pu[:], func=AF.Tanh,
                                 bias=sentb_sb[:, m:m + 1])
        pscs = psA.tile([1, NS], F32, tag="psc")
        for k in range(5):
            nc.tensor.matmul(out=pscs[:], lhsT=sproj_sb[:, k:k + 1], rhs=usT[:, k, :],
                            start=(k == 0), stop=(k == 4))
        scs = small.tile([1, NS], F32, tag="scs")
        nc.vector.tensor_copy(out=scs[:], in_=pscs[:])
        nc.sync.dma_start(out=t["scs_d"][:], in_=scs[:])
        sc2s = small.tile([ND, S], F32, tag="sc2s")
        nc.sync.dma_start(out=sc2s[:], in_=_ap(t["scs_d"], 0, [[16, ND], [1, S]]))
        mxs = small.tile([ND, 1], F32, tag="mxs")
        nc.vector.tensor_reduce(out=mxs[:], in_=sc2s[:], axis=AX.X, op=ALU.max,
                                negate=True)
        exs = small.tile([ND, S], F32, tag="exs")
        dens = small.tile([ND, 1], F32, tag="dens")
        nc.scalar.activation(out=exs[:], in_=sc2s[:], func=AF.Exp, bias=mxs[:],
                             accum_out=dens[:])
        rcs = small.tile([ND, 1], F32, tag="rcs")
        nc.vector.reciprocal(out=rcs[:], in_=dens[:])
        alps_bf = small.tile([ND, S], BF16, tag="alps_bf")
        nc.vector.tensor_scalar_mul(out=alps_bf[:], in0=exs[:], scalar1=rcs[:])
        # amats [64, 4]: column d rows d*16:(d+1)*16 = alps_bf[d, :]
        amats = small.tile([64, ND], BF16, tag="amats")
        nc.gpsimd.memset(amats[:], 0.0)
        nc.sync.dma_start(out=t["alps_d"][:], in_=alps_bf[:])
        for d in range(ND):
            nc.sync.dma_start(out=amats[d * S:(d + 1) * S, d:d + 1],
                              in_=t["alps_d"][d])
        hbs = st.tile([64, 512], BF16)
        for c in range(4):
            ptb = pst.tile([128, 128], BF16, tag="pst")
            nc.tensor.transpose(out=ptb[:64, :], in_=hs_all[:, c, :], identity=ident[:])
            nc.vector.tensor_copy(out=hbs[:, c * 128:(c + 1) * 128], in_=ptb[:64, :])
        pdoc = psA.tile([ND, 512], F32, tag="pu")
        nc.tensor.matmul(out=pdoc[:], lhsT=amats[:], rhs=hbs[:], start=True, stop=True)
        doc_sb = st.tile([ND, 512], BF16)
        nc.vector.tensor_copy(out=doc_sb[:], in_=pdoc[:])

        # feats^T [128, 8, ND]
        featsT = st.tile([128, 8, ND], BF16)
        for c in range(4):
            ptf = pst.tile([128, 128], BF16, tag="pst")
            nc.tensor.transpose(out=ptf[:, :ND], in_=doc_sb[:, c * 128:(c + 1) * 128],
                                identity=ident[:ND, :ND])
            nc.vector.tensor_copy(out=featsT[:, c, :], in_=ptf[:, :ND])
        nc.vector.tensor_copy(
            out=featsT[:, 4:6, :],
            in_=_ap(hs_all[:, 0, :], S - 1, [[256, 128], [64, 2], [16, ND]]))
        nc.vector.tensor_copy(
            out=featsT[:, 6:8, :],
            in_=_ap(hs_all[:, 2, :], 0, [[256, 128], [64, 2], [16, ND]]))
        pout = psA.tile([POL, ND], F32, tag="psc")
        for k in range(8):
            nc.tensor.matmul(out=pout[:], lhsT=dW_sb[:, k, :], rhs=featsT[:, k, :],
                            start=(k == 0), stop=(k == 7))
        yf = small.tile([POL, ND], F32, tag="yf")
        nc.scalar.activation(out=yf[:], in_=pout[:], func=AF.Identity, bias=db_sb[:])
        nc.sync.dma_start(out=_ap(t["y_out"][:], 0, [[1, POL], [POL, ND]]), in_=yf[:])
        ps_ctx.close()


_NC_CACHE = None


def _get_nc():
    global _NC_CACHE
    if _NC_CACHE is None:
        _NC_CACHE = _build_nc()
    return _NC_CACHE


def _prep_host(inputs):
    gpm = GPERM
    wihs, whhs = [], []
    for wih, b in [(inputs["wWih_f"], inputs["wb_f"]),
                   (inputs["wWih_b"], inputs["wb_b"])]:
        wt = np.zeros((512, 1024), np.float32)
        wp = np.asarray(wih, np.float32)[gpm]      # [1024, 400]
        wt[0:300, :] = wp[:, 0:300].T
        wt[384:484, :] = wp[:, 300:400].T
        wt[511, :] = np.asarray(b, np.float32)[gpm]
        wihs.append(wt.reshape(4, 128, 1024))
    for whh in [inputs["wWhh_f"], inputs["wWhh_b"]]:
        whhs.append(np.asarray(whh, np.float32)[gpm].T.reshape(2, 128, 1024))
    swihs, swhhs = [], []
    for wih, b in [(inputs["sWih_f"], inputs["sb_f"]),
                   (inputs["sWih_b"], inputs["sb_b"])]:
        wt = np.zeros((640, 1024), np.float32)
        wp = np.asarray(wih, np.float32)[gpm]      # [1024, 612]
        wt[0:612, :] = wp.T
        wt[639, :] = np.asarray(b, np.float32)[gpm]
        swihs.append(wt.reshape(5, 128, 1024))
    for whh in [inputs["sWhh_f"], inputs["sWhh_b"]]:
        swhhs.append(np.asarray(whh, np.float32)[gpm].T.reshape(2, 128, 1024))

    def padW(w):
        o = np.zeros((640, 640), np.float32)
        o[:612, :612] = np.asarray(w, np.float32)
        return o.reshape(5, 128, 640)

    def padv(v, chunks):
        o = np.zeros((chunks * 128,), np.float32)
        o[:len(v)] = np.asarray(v, np.float32)
        return np.ascontiguousarray(o.reshape(chunks, 128).T)

    m2 = np.zeros((128, 64), np.float32)
    for p in range(128):
        m2[p, p % 64] = 1.0

    def pmajor(a):
        # [..., 128, X] -> [128, ..., X] contiguous
        a = np.asarray(a)
        nd = a.ndim
        perm = (nd - 2,) + tuple(range(nd - 2)) + (nd - 1,)
        return np.ascontiguousarray(a.transpose(perm))

    shared = {
        "emb_t": _bf(inputs["emb"]),
        "wpe_t": _bf(inputs["wpos_emb"]),
        "spe_t": _bf(inputs["spos_emb"]),
        "wihT": _bf(pmajor(np.stack(wihs))),           # [128, 2, 4, 1024]
        "whhT": _bf(pmajor(np.stack(whhs))),           # [128, 2, 2, 1024]
        "swihT": _bf(pmajor(np.stack(swihs))),         # [128, 2, 5, 1024]
        "swhhT": _bf(pmajor(np.stack(swhhs))),         # [128, 2, 2, 1024]
        "wordW": _bf(pmajor(padW(inputs["word_W"]))),  # [128, 5, 640]
        "wordb": padv(inputs["word_bias"], 5).astype(np.float32),
        "wproj": _bf(padv(inputs["word_proj"], 5)),
        "sentW": _bf(pmajor(padW(inputs["sent_W"]))),
        "sentb": padv(inputs["sent_bias"], 5).astype(np.float32),
        "sproj": _bf(padv(inputs["sent_proj"], 5)),
        "denseWT": _bf(pmajor(np.ascontiguousarray(
            np.asarray(inputs["dense_W"], np.float32).T).reshape(8, 128, POL))),
        "denseb": np.asarray(inputs["dense_b"], np.float32).reshape(POL, 1),
        "id128": _bf(np.eye(128)),
        "m2": _bf(m2),
    }

    toks = np.asarray(inputs["text_raw_indices"], np.int64).reshape(B, S, L)
    wpos = np.asarray(inputs["word_position"], np.int64).reshape(B, S, L)
    spos = np.asarray(inputs["segment_position"], np.int64).reshape(B, S)
    in_maps = []
    for c in range(NCORES):
        tk = toks[c * ND:(c + 1) * ND].reshape(NS, L)   # [64 s, 64 t]
        wp = wpos[c * ND:(c + 1) * ND].reshape(NS, L)
        sp = spos[c * ND:(c + 1) * ND]                  # [4, 16]
        m = dict(shared)
        idx = np.stack([np.ascontiguousarray(tk.T).reshape(NTILE, 128).T,
                        np.ascontiguousarray(wp.T).reshape(NTILE, 128).T],
                       axis=1)                          # [128, 2, NTILE]
        m["idx"] = np.ascontiguousarray(idx).astype(np.int32)
        m["sp_idx"] = sp.reshape(NS, 1).astype(np.int32)
        in_maps.append(m)
    return in_maps


def kernel(**inputs):
    nc = _get_nc()
    in_maps = _prep_host(inputs)
    res = run_bass_kernel_spmd(nc, in_maps, list(range(NCORES)))
    out = np.concatenate([res.results[c]["y"] for c in range(NCORES)], axis=0)
    return out.astype(np.float32)
